# revision 3
# baseline (speedup 1.0000x reference)
"""Multi-head attention (B=4, S=2048, D=1024, H=16, causal) on 8 trn2 cores.

Sharding: core c -> (batch b = c//2, head-group hg = c%2 of 8 heads).
Host pre-transposes/casts activations to bf16 [D, S] and slices weights;
device computes a partial [S, D] output (its head-group's contribution
through the output projection); host sums the pair per batch and adds bo.
"""

import numpy as np
import ml_dtypes

import concourse.bacc as bacc
import concourse.bass as bass
import concourse.mybir as mybir
import concourse.tile as tile
from concourse.bass_utils import run_bass_kernel_spmd

B, S, D, H = 4, 2048, 1024, 16
DH = D // H          # 64
HG = H // 2          # 8 heads per core
DG = HG * DH         # 512 dims per core
N_CORES = 8

BF16 = mybir.dt.bfloat16
F32 = mybir.dt.float32

ST = S // 128        # 16 seq tiles of 128
QB = S // 512        # 4 query blocks of 512
KT = D // 128        # 8 contraction tiles for the input projections
VBLK = DH + 1        # 65: per-head v columns + ones column
AF = mybir.ActivationFunctionType
ALU = mybir.AluOpType


def build_program():
    nc = bacc.Bacc("TRN2", target_bir_lowering=False, debug=False,
                   num_devices=N_CORES)

    xq = nc.declare_dram_parameter("xq", [D, S], BF16, isOutput=False)
    xk = nc.declare_dram_parameter("xk", [D, S], BF16, isOutput=False)
    xv = nc.declare_dram_parameter("xv", [D, S], BF16, isOutput=False)
    wq = nc.declare_dram_parameter("wq", [D, DG], BF16, isOutput=False)
    wk = nc.declare_dram_parameter("wk", [D, DG], BF16, isOutput=False)
    wv = nc.declare_dram_parameter("wv", [D, DG], BF16, isOutput=False)
    wo = nc.declare_dram_parameter("wo", [DG, D], BF16, isOutput=False)
    bq = nc.declare_dram_parameter("bq", [DG, 1], F32, isOutput=False)
    bk = nc.declare_dram_parameter("bk", [DG, 1], F32, isOutput=False)
    bv = nc.declare_dram_parameter("bv", [DG, 1], F32, isOutput=False)
    out = nc.declare_dram_parameter("out", [S, D], F32, isOutput=True)

    with tile.TileContext(nc) as tc:
        with (
            tc.tile_pool(name="persist", bufs=1) as persist,
            tc.tile_pool(name="xin", bufs=10) as xin,
            tc.tile_pool(name="xvin", bufs=8) as xvin,
            tc.tile_pool(name="exp", bufs=6) as expp,
            tc.tile_pool(name="small", bufs=4) as small,
            tc.tile_pool(name="outp", bufs=3) as outp,
            tc.tile_pool(name="ps512", bufs=2, space="PSUM") as ps512,
            tc.tile_pool(name="pssc", bufs=3, space="PSUM") as pssc,
            tc.tile_pool(name="psav", bufs=2, space="PSUM") as psav,
        ):
            # ---- resident weights / constants ----
            wq_sb = persist.tile([128, KT * DG], BF16, tag="wq")
            wk_sb = persist.tile([128, KT * DG], BF16, tag="wk")
            wv_sb = persist.tile([128, KT * DG], BF16, tag="wv")
            for j in range(KT):
                nc.sync.dma_start(wq_sb[:, bass.ts(j, DG)], wq[bass.ts(j, 128), :])
                nc.sync.dma_start(wk_sb[:, bass.ts(j, DG)], wk[bass.ts(j, 128), :])
                nc.sync.dma_start(wv_sb[:, bass.ts(j, DG)], wv[bass.ts(j, 128), :])
            wo_sb = persist.tile([128, 4 * D], BF16, tag="wo")
            for j in range(4):
                nc.sync.dma_start(wo_sb[:, bass.ts(j, D)], wo[bass.ts(j, 128), :])
            bq_sb = persist.tile([128, 4], F32, tag="bq")
            bk_sb = persist.tile([128, 4], F32, tag="bk")
            bv_sb = persist.tile([128, 4], F32, tag="bv")
            for t in range(4):
                nc.sync.dma_start(bq_sb[:, t : t + 1], bq[bass.ts(t, 128), :])
                nc.sync.dma_start(bk_sb[:, t : t + 1], bk[bass.ts(t, 128), :])
                nc.sync.dma_start(bv_sb[:, t : t + 1], bv[bass.ts(t, 128), :])

            # 4 causal mask tiles: mask_r[p, f] = 1.0 if p + 128*r <= f else 0
            masks = persist.tile([128, 4 * 512], BF16, tag="masks")
            nc.gpsimd.memset(masks[:], 1.0)
            for r in range(4):
                nc.gpsimd.affine_select(
                    out=masks[:, bass.ts(r, 512)],
                    in_=masks[:, bass.ts(r, 512)],
                    compare_op=ALU.is_ge,
                    fill=0.0,
                    base=-128 * r,
                    pattern=[[1, 512]],
                    channel_multiplier=-1,
                )
            ones64 = persist.tile([1, DH], F32, tag="ones64")
            nc.gpsimd.memset(ones64[:], 1.0)

            # persistent activations
            qt = [persist.tile([128, S], BF16, tag=f"qt{t}", name=f"qt{t}") for t in range(4)]
            kt = [persist.tile([128, S], BF16, tag=f"kt{t}", name=f"kt{t}") for t in range(4)]
            v_sb = persist.tile([128, ST * HG * VBLK], BF16, tag="v_sb")
            ao = [persist.tile([128, S], BF16, tag=f"ao{t}", name=f"ao{t}") for t in range(4)]

            # ones columns of v blocks (written before the v copies below)
            v_view = v_sb[:].rearrange("p (s h c) -> p s h c", s=ST, h=HG, c=VBLK)
            nc.gpsimd.memset(v_view[:, :, :, DH : DH + 1], 1.0)

            # ---- K / Q projections: out[t][:, n*512:+512] over 8 k-tiles ----
            for n in range(QB):
                xk_t = []
                xq_t = []
                for j in range(KT):
                    tk = xin.tile([128, 512], BF16, tag="xkq")
                    nc.sync.dma_start(tk[:], xk[bass.ts(j, 128), bass.ts(n, 512)])
                    xk_t.append(tk)
                for j in range(KT):
                    tq = xin.tile([128, 512], BF16, tag="xkq")
                    nc.sync.dma_start(tq[:], xq[bass.ts(j, 128), bass.ts(n, 512)])
                    xq_t.append(tq)
                for t in range(4):
                    ps = ps512.tile([128, 512], F32, tag="mm512")
                    for j in range(KT):
                        nc.tensor.matmul(
                            ps[:],
                            wk_sb[:, j * DG + t * 128 : j * DG + (t + 1) * 128],
                            xk_t[j][:],
                            start=(j == 0),
                            stop=(j == KT - 1),
                        )
                    nc.vector.tensor_scalar_add(
                        kt[t][:, bass.ts(n, 512)], ps[:], bk_sb[:, t : t + 1]
                    )
                for t in range(4):
                    ps = ps512.tile([128, 512], F32, tag="mm512")
                    for j in range(KT):
                        nc.tensor.matmul(
                            ps[:],
                            wq_sb[:, j * DG + t * 128 : j * DG + (t + 1) * 128],
                            xq_t[j][:],
                            start=(j == 0),
                            stop=(j == KT - 1),
                        )
                    # (q + bq) * (1/sqrt(DH)) fused: (ps add bq) mult 0.125
                    nc.vector.tensor_scalar(
                        qt[t][:, bass.ts(n, 512)], ps[:],
                        bq_sb[:, t : t + 1], 0.125, ALU.add, ALU.mult,
                    )

            # ---- V projection -> natural layout [s, dims] w/ ones cols ----
            xv_t = []
            for j in range(KT):
                tv = xvin.tile([128, S], BF16, tag="xv")
                nc.sync.dma_start(tv[:], xv[bass.ts(j, 128), :])
                xv_t.append(tv)
            for s in range(ST):
                ps = ps512.tile([128, 512], F32, tag="mm512")
                for j in range(KT):
                    nc.tensor.matmul(
                        ps[:],
                        xv_t[j][:, bass.ts(s, 128)],
                        wv_sb[:, bass.ts(j, DG)],
                        start=(j == 0),
                        stop=(j == KT - 1),
                    )
                dst = v_view[:, s, :, 0:DH]
                src = ps[:].rearrange("p (h c) -> p h c", c=DH)
                nc.vector.tensor_copy(dst, src)

            # ---- attention + output projection, per 512-wide q block ----
            for n in range(QB):
                nk = 4 * (n + 1)
                for h in range(HG):
                    t, r = h // 2, h % 2
                    q_ap = qt[t][r * DH : (r + 1) * DH, bass.ts(n, 512)]
                    av = psav.tile([VBLK, 512], F32, tag="av")
                    for j in range(nk):
                        sc = pssc.tile([128, 512], F32, tag="sc")
                        nc.tensor.matmul(
                            sc[:],
                            kt[t][r * DH : (r + 1) * DH, bass.ts(j, 128)],
                            q_ap,
                            start=True,
                            stop=True,
                        )
                        ex = expp.tile([128, 512], BF16, tag="ex")
                        nc.scalar.activation(ex[:], sc[:], AF.Exp)
                        if j >= 4 * n:
                            nc.vector.tensor_mul(
                                ex[:], ex[:], masks[:, bass.ts(j - 4 * n, 512)]
                            )
                        nc.tensor.matmul(
                            av[:],
                            v_sb[:, (s_off := j * HG * VBLK + h * VBLK) : s_off + VBLK],
                            ex[:],
                            start=(j == 0),
                            stop=(j == nk - 1),
                        )
                    recip = small.tile([1, 512], F32, tag="recip")
                    nc.vector.reciprocal(recip[:], av[DH : DH + 1, :])
                    bc = pssc.tile([DH, 512], F32, tag="sc")
                    nc.tensor.matmul(bc[:], ones64[:], recip[:], start=True, stop=True)
                    rb = small.tile([DH, 512], F32, tag="rb")
                    nc.scalar.copy(rb[:], bc[:])
                    dst = ao[t][r * DH : (r + 1) * DH, bass.ts(n, 512)]
                    nc.vector.tensor_mul(dst, av[0:DH, :], rb[:])
                    nc.vector.tensor_scalar_add(
                        dst, dst, bv_sb[r * DH : (r + 1) * DH, t : t + 1]
                    )
                # output projection rows finished by this q block
                for s in range(4 * n, 4 * n + 4):
                    for m in range(2):
                        po = ps512.tile([128, 512], F32, tag="mm512")
                        for kk in range(4):
                            nc.tensor.matmul(
                                po[:],
                                ao[kk][:, bass.ts(s, 128)],
                                wo_sb[:, kk * D + m * 512 : kk * D + (m + 1) * 512],
                                start=(kk == 0),
                                stop=(kk == 3),
                            )
                        ob = outp.tile([128, 512], F32, tag="ob")
                        nc.vector.tensor_copy(ob[:], po[:])
                        nc.sync.dma_start(out[bass.ts(s, 128), bass.ts(m, 512)], ob[:])

    nc.compile()
    return nc


_NC = None


def _get_program():
    global _NC
    if _NC is None:
        _NC = build_program()
    return _NC


def make_in_maps(query, key, value, Wq, bq, Wk, bk, Wv, bv, Wo):
    bf = ml_dtypes.bfloat16
    in_maps = []
    xqs = [np.ascontiguousarray(query[b].T).astype(bf) for b in range(B)]
    xks = [np.ascontiguousarray(key[b].T).astype(bf) for b in range(B)]
    xvs = [np.ascontiguousarray(value[b].T).astype(bf) for b in range(B)]
    for c in range(N_CORES):
        b, hg = c // 2, c % 2
        sl = slice(hg * DG, (hg + 1) * DG)
        in_maps.append({
            "xq": xqs[b], "xk": xks[b], "xv": xvs[b],
            "wq": np.ascontiguousarray(Wq[sl, :].T).astype(bf),
            "wk": np.ascontiguousarray(Wk[sl, :].T).astype(bf),
            "wv": np.ascontiguousarray(Wv[sl, :].T).astype(bf),
            "wo": np.ascontiguousarray(Wo[:, sl].T).astype(bf),
            "bq": np.asarray(bq[sl], np.float32).reshape(DG, 1),
            "bk": np.asarray(bk[sl], np.float32).reshape(DG, 1),
            "bv": np.asarray(bv[sl], np.float32).reshape(DG, 1),
        })
    return in_maps


def combine_outputs(results, bo):
    out = np.empty((B, S, D), np.float32)
    for b in range(B):
        out[b] = results[2 * b]["out"] + results[2 * b + 1]["out"]
        out[b] += np.asarray(bo, np.float32)[None, :]
    return out


def kernel(query, key, value, mask, Wq, bq, Wk, bk, Wv, bv, Wo, bo):
    # mask is the causal tril mask from the reference problem; causality is
    # implemented directly in the device kernel.
    nc = _get_program()
    in_maps = make_in_maps(
        np.asarray(query, np.float32), np.asarray(key, np.float32),
        np.asarray(value, np.float32), np.asarray(Wq, np.float32),
        np.asarray(bq, np.float32), np.asarray(Wk, np.float32),
        np.asarray(bk, np.float32), np.asarray(Wv, np.float32),
        np.asarray(bv, np.float32), np.asarray(Wo, np.float32),
    )
    res = run_bass_kernel_spmd(nc, in_maps, list(range(N_CORES)))
    return combine_outputs(res.results, np.asarray(bo, np.float32))


# revision 5
# speedup vs baseline: 116.7707x; 116.7707x over previous
"""Multi-head attention (B=4, S=2048, D=1024, H=16, causal) on 8 trn2 cores.

Sharding: core c -> (batch b = c//2, head-group hg = c%2 of 8 heads).
Host pre-transposes/casts activations to bf16 [D, S] and slices weights;
device computes a partial [S, D] output (its head-group's contribution
through the output projection); host sums the pair per batch and adds bo.
"""

import numpy as np
import ml_dtypes

import concourse.bacc as bacc
import concourse.bass as bass
import concourse.mybir as mybir
import concourse.tile as tile
from concourse.bass_utils import run_bass_kernel_spmd

B, S, D, H = 4, 2048, 1024, 16
DH = D // H          # 64
HG = H // 2          # 8 heads per core
DG = HG * DH         # 512 dims per core
N_CORES = 8

BF16 = mybir.dt.bfloat16
F32 = mybir.dt.float32

ST = S // 128        # 16 seq tiles of 128
QB = S // 512        # 4 query blocks of 512
KT = D // 128        # 8 contraction tiles for the input projections
VBLK = DH + 1        # 65: per-head v columns + ones column
AF = mybir.ActivationFunctionType
ALU = mybir.AluOpType


def build_program(loop_r=0):
    """loop_r > 0 builds a measurement variant that repeats the whole body
    loop_r times inside an on-device loop (for timing via slope)."""
    nc = bacc.Bacc("TRN2", target_bir_lowering=False, debug=False,
                   num_devices=N_CORES)

    xq = nc.declare_dram_parameter("xq", [D, S], BF16, isOutput=False)
    xk = nc.declare_dram_parameter("xk", [D, S], BF16, isOutput=False)
    xv = nc.declare_dram_parameter("xv", [D, S], BF16, isOutput=False)
    wq = nc.declare_dram_parameter("wq", [D, DG], BF16, isOutput=False)
    wk = nc.declare_dram_parameter("wk", [D, DG], BF16, isOutput=False)
    wv = nc.declare_dram_parameter("wv", [D, DG], BF16, isOutput=False)
    wo = nc.declare_dram_parameter("wo", [DG, D], BF16, isOutput=False)
    bq = nc.declare_dram_parameter("bq", [DG, 1], F32, isOutput=False)
    bk = nc.declare_dram_parameter("bk", [DG, 1], F32, isOutput=False)
    bv = nc.declare_dram_parameter("bv", [DG, 1], F32, isOutput=False)
    out = nc.declare_dram_parameter("out", [S, D], F32, isOutput=True)

    with tile.TileContext(nc) as tc:
        with (
            tc.tile_pool(name="persist", bufs=1) as persist,
            tc.tile_pool(name="xin", bufs=10) as xin,
            tc.tile_pool(name="xvin", bufs=8) as xvin,
            tc.tile_pool(name="exp", bufs=6) as expp,
            tc.tile_pool(name="small", bufs=4) as small,
            tc.tile_pool(name="outp", bufs=3) as outp,
            tc.tile_pool(name="ps512", bufs=2, space="PSUM") as ps512,
            tc.tile_pool(name="pssc", bufs=3, space="PSUM") as pssc,
            tc.tile_pool(name="psav", bufs=2, space="PSUM") as psav,
        ):
            import contextlib
            loop_cm = tc.For_i(0, loop_r, 1) if loop_r else contextlib.nullcontext()
            with loop_cm:
                emit_body(nc, tc, locals())
    nc.compile()
    return nc


def emit_body(nc, tc, pools):
    persist = pools["persist"]; xin = pools["xin"]; xvin = pools["xvin"]
    expp = pools["expp"]; small = pools["small"]; outp = pools["outp"]
    ps512 = pools["ps512"]; pssc = pools["pssc"]; psav = pools["psav"]
    xq = pools["xq"]; xk = pools["xk"]; xv = pools["xv"]
    wq = pools["wq"]; wk = pools["wk"]; wv = pools["wv"]; wo = pools["wo"]
    bq = pools["bq"]; bk = pools["bk"]; bv = pools["bv"]; out = pools["out"]
    if True:
        if True:
            # ---- resident weights / constants ----
            wq_sb = persist.tile([128, KT * DG], BF16, tag="wq")
            wk_sb = persist.tile([128, KT * DG], BF16, tag="wk")
            wv_sb = persist.tile([128, KT * DG], BF16, tag="wv")
            for j in range(KT):
                nc.sync.dma_start(wq_sb[:, bass.ts(j, DG)], wq[bass.ts(j, 128), :])
                nc.sync.dma_start(wk_sb[:, bass.ts(j, DG)], wk[bass.ts(j, 128), :])
                nc.sync.dma_start(wv_sb[:, bass.ts(j, DG)], wv[bass.ts(j, 128), :])
            wo_sb = persist.tile([128, 4 * D], BF16, tag="wo")
            for j in range(4):
                nc.sync.dma_start(wo_sb[:, bass.ts(j, D)], wo[bass.ts(j, 128), :])
            bq_sb = persist.tile([128, 4], F32, tag="bq")
            bk_sb = persist.tile([128, 4], F32, tag="bk")
            bv_sb = persist.tile([128, 4], F32, tag="bv")
            for t in range(4):
                nc.sync.dma_start(bq_sb[:, t : t + 1], bq[bass.ts(t, 128), :])
                nc.sync.dma_start(bk_sb[:, t : t + 1], bk[bass.ts(t, 128), :])
                nc.sync.dma_start(bv_sb[:, t : t + 1], bv[bass.ts(t, 128), :])

            # 4 causal mask tiles: mask_r[p, f] = 1.0 if p + 128*r <= f else 0
            masks = persist.tile([128, 4 * 512], BF16, tag="masks")
            nc.gpsimd.memset(masks[:], 1.0)
            for r in range(4):
                nc.gpsimd.affine_select(
                    out=masks[:, bass.ts(r, 512)],
                    in_=masks[:, bass.ts(r, 512)],
                    compare_op=ALU.is_ge,
                    fill=0.0,
                    base=-128 * r,
                    pattern=[[1, 512]],
                    channel_multiplier=-1,
                )
            ones64 = persist.tile([1, DH], F32, tag="ones64")
            nc.gpsimd.memset(ones64[:], 1.0)

            # persistent activations
            qt = [persist.tile([128, S], BF16, tag=f"qt{t}", name=f"qt{t}") for t in range(4)]
            kt = [persist.tile([128, S], BF16, tag=f"kt{t}", name=f"kt{t}") for t in range(4)]
            v_sb = persist.tile([128, ST * HG * VBLK], BF16, tag="v_sb")
            ao = [persist.tile([128, S], BF16, tag=f"ao{t}", name=f"ao{t}") for t in range(4)]

            # ones columns of v blocks (written before the v copies below)
            v_view = v_sb[:].rearrange("p (s h c) -> p s h c", s=ST, h=HG, c=VBLK)
            nc.gpsimd.memset(v_view[:, :, :, DH : DH + 1], 1.0)

            # ---- K / Q projections: out[t][:, n*512:+512] over 8 k-tiles ----
            for n in range(QB):
                xk_t = []
                xq_t = []
                for j in range(KT):
                    tk = xin.tile([128, 512], BF16, tag="xkq")
                    nc.sync.dma_start(tk[:], xk[bass.ts(j, 128), bass.ts(n, 512)])
                    xk_t.append(tk)
                for j in range(KT):
                    tq = xin.tile([128, 512], BF16, tag="xkq")
                    nc.sync.dma_start(tq[:], xq[bass.ts(j, 128), bass.ts(n, 512)])
                    xq_t.append(tq)
                for t in range(4):
                    ps = ps512.tile([128, 512], F32, tag="mm512")
                    for j in range(KT):
                        nc.tensor.matmul(
                            ps[:],
                            wk_sb[:, j * DG + t * 128 : j * DG + (t + 1) * 128],
                            xk_t[j][:],
                            start=(j == 0),
                            stop=(j == KT - 1),
                        )
                    nc.vector.tensor_scalar_add(
                        kt[t][:, bass.ts(n, 512)], ps[:], bk_sb[:, t : t + 1]
                    )
                for t in range(4):
                    ps = ps512.tile([128, 512], F32, tag="mm512")
                    for j in range(KT):
                        nc.tensor.matmul(
                            ps[:],
                            wq_sb[:, j * DG + t * 128 : j * DG + (t + 1) * 128],
                            xq_t[j][:],
                            start=(j == 0),
                            stop=(j == KT - 1),
                        )
                    # (q + bq) * (1/sqrt(DH)) fused: (ps add bq) mult 0.125
                    nc.vector.tensor_scalar(
                        qt[t][:, bass.ts(n, 512)], ps[:],
                        bq_sb[:, t : t + 1], 0.125, ALU.add, ALU.mult,
                    )

            # ---- V projection -> natural layout [s, dims] w/ ones cols ----
            xv_t = []
            for j in range(KT):
                tv = xvin.tile([128, S], BF16, tag="xv")
                nc.sync.dma_start(tv[:], xv[bass.ts(j, 128), :])
                xv_t.append(tv)
            for s in range(ST):
                ps = ps512.tile([128, 512], F32, tag="mm512")
                for j in range(KT):
                    nc.tensor.matmul(
                        ps[:],
                        xv_t[j][:, bass.ts(s, 128)],
                        wv_sb[:, bass.ts(j, DG)],
                        start=(j == 0),
                        stop=(j == KT - 1),
                    )
                dst = v_view[:, s, :, 0:DH]
                src = ps[:].rearrange("p (h c) -> p h c", c=DH)
                nc.vector.tensor_copy(dst, src)

            # ---- attention + output projection, per 512-wide q block ----
            for n in range(QB):
                nk = 4 * (n + 1)
                for h in range(HG):
                    t, r = h // 2, h % 2
                    q_ap = qt[t][r * DH : (r + 1) * DH, bass.ts(n, 512)]
                    av = psav.tile([VBLK, 512], F32, tag="av")
                    for j in range(nk):
                        sc = pssc.tile([128, 512], F32, tag="sc")
                        nc.tensor.matmul(
                            sc[:],
                            kt[t][r * DH : (r + 1) * DH, bass.ts(j, 128)],
                            q_ap,
                            start=True,
                            stop=True,
                        )
                        ex = expp.tile([128, 512], BF16, tag="ex")
                        nc.scalar.activation(ex[:], sc[:], AF.Exp)
                        if j >= 4 * n:
                            nc.vector.tensor_mul(
                                ex[:], ex[:], masks[:, bass.ts(j - 4 * n, 512)]
                            )
                        nc.tensor.matmul(
                            av[:],
                            v_sb[:, (s_off := j * HG * VBLK + h * VBLK) : s_off + VBLK],
                            ex[:],
                            start=(j == 0),
                            stop=(j == nk - 1),
                        )
                    recip = small.tile([1, 512], F32, tag="recip")
                    nc.vector.reciprocal(recip[:], av[DH : DH + 1, :])
                    bc = pssc.tile([DH, 512], F32, tag="sc")
                    nc.tensor.matmul(bc[:], ones64[:], recip[:], start=True, stop=True)
                    rb = small.tile([DH, 512], F32, tag="rb")
                    nc.scalar.copy(rb[:], bc[:])
                    dst = ao[t][r * DH : (r + 1) * DH, bass.ts(n, 512)]
                    nc.vector.tensor_mul(dst, av[0:DH, :], rb[:])
                    nc.vector.tensor_scalar_add(
                        dst, dst, bv_sb[r * DH : (r + 1) * DH, t : t + 1]
                    )
                # output projection rows finished by this q block
                for s in range(4 * n, 4 * n + 4):
                    for m in range(2):
                        po = ps512.tile([128, 512], F32, tag="mm512")
                        for kk in range(4):
                            nc.tensor.matmul(
                                po[:],
                                ao[kk][:, bass.ts(s, 128)],
                                wo_sb[:, kk * D + m * 512 : kk * D + (m + 1) * 512],
                                start=(kk == 0),
                                stop=(kk == 3),
                            )
                        ob = outp.tile([128, 512], F32, tag="ob")
                        nc.vector.tensor_copy(ob[:], po[:])
                        nc.sync.dma_start(out[bass.ts(s, 128), bass.ts(m, 512)], ob[:])


_NC = None


def _get_program():
    global _NC
    if _NC is None:
        _NC = build_program()
    return _NC


def make_in_maps(query, key, value, Wq, bq, Wk, bk, Wv, bv, Wo):
    bf = ml_dtypes.bfloat16
    in_maps = []
    xqs = [np.ascontiguousarray(query[b].T).astype(bf) for b in range(B)]
    xks = [np.ascontiguousarray(key[b].T).astype(bf) for b in range(B)]
    xvs = [np.ascontiguousarray(value[b].T).astype(bf) for b in range(B)]
    for c in range(N_CORES):
        b, hg = c // 2, c % 2
        sl = slice(hg * DG, (hg + 1) * DG)
        in_maps.append({
            "xq": xqs[b], "xk": xks[b], "xv": xvs[b],
            "wq": np.ascontiguousarray(Wq[sl, :].T).astype(bf),
            "wk": np.ascontiguousarray(Wk[sl, :].T).astype(bf),
            "wv": np.ascontiguousarray(Wv[sl, :].T).astype(bf),
            "wo": np.ascontiguousarray(Wo[:, sl].T).astype(bf),
            "bq": np.asarray(bq[sl], np.float32).reshape(DG, 1),
            "bk": np.asarray(bk[sl], np.float32).reshape(DG, 1),
            "bv": np.asarray(bv[sl], np.float32).reshape(DG, 1),
        })
    return in_maps


def combine_outputs(results, bo):
    out = np.empty((B, S, D), np.float32)
    for b in range(B):
        out[b] = results[2 * b]["out"] + results[2 * b + 1]["out"]
        out[b] += np.asarray(bo, np.float32)[None, :]
    return out


def kernel(query, key, value, mask, Wq, bq, Wk, bk, Wv, bv, Wo, bo):
    # mask is the causal tril mask from the reference problem; causality is
    # implemented directly in the device kernel.
    nc = _get_program()
    in_maps = make_in_maps(
        np.asarray(query, np.float32), np.asarray(key, np.float32),
        np.asarray(value, np.float32), np.asarray(Wq, np.float32),
        np.asarray(bq, np.float32), np.asarray(Wk, np.float32),
        np.asarray(bk, np.float32), np.asarray(Wv, np.float32),
        np.asarray(bv, np.float32), np.asarray(Wo, np.float32),
    )
    res = run_bass_kernel_spmd(nc, in_maps, list(range(N_CORES)))
    return combine_outputs(res.results, np.asarray(bo, np.float32))
